# revision 1
# baseline (speedup 1.0000x reference)
"""Expected Calibration Error (histogram binning) on 8 Trainium2 NeuronCores.

kernel(outputs [1e6,100] f32, targets [1e6] int) -> f32 scalar, matching the
reference softmax/argmax/10-bin ECE. Data-parallel over the batch; each core
streams its 50 MB shard once from HBM (~140 us/core roofline at 358 GB/s).

Layout trick: every per-row ECE statistic (row max, row sum of exp, true-class
prob, argmax==target, bin membership) is invariant to a cyclic rotation of the
row's class axis. The host rolls each row left by its target class — the
true-class logit lands in column 0 for every row — and packs rows densely onto
a [8 cores, 128 partitions, 980 rows] grid (0.35% padding; pad rows are
[-300, 0, ...], whose exp underflows to exactly 0 so p == 0 and the
reference's own p > 0 rule excludes them). Rows are dealt sorted-by-class only
to make the host-side rolling two contiguous slice copies per class.

Device, per chunk of 49 rows/partition ([128, 49, 100] tile):
  - DVE:  segmented reduce_max over classes (for argmax-equality correctness)
  - ACT:  exp of the whole tile; strided copy of column 0 (true-class logit)
  - PE:   row-sum-of-exp for 79 classes via accumulating identity matmuls
  - DVE:  segmented reduce_add for the remaining 21 classes (engine balance)
Finish (two interleaved slabs): p = exp(x_t) * (1/s) (DVE reciprocal),
correct = (x_t == m), z = p * correct (GPSIMD), then per boundary b of
linspace(0,1,11), fused full-tile-scan + per-partition accumulate ops:
  C_b = count(p > b)          (DVE  is_gt + accumulate)
  R_b = sum(relu(p - b))      (ACT  relu with bias=-b + accumulate)
  Z_b = count(z > b)          (DVE  is_gt + accumulate)
Host: sum the 8x128 partials in f64; SP_b = R_b + b*C_b recovers the
cumulative sum of confidences; adjacent differences give the 10 bins; finish
the ECE scalar exactly as the reference does. All four engines plus DMA land
within ~10% of the per-core HBM roofline (cost model: ~170 us vs ~140 us DMA).
"""

import os
import sys
import tempfile

import numpy as np

if "/opt/trn_rl_repo" not in sys.path:
    sys.path.insert(0, "/opt/trn_rl_repo")

# Persistent jax/PJRT executable cache (includes the compiled NEFF): makes
# repeat invocations and the subprocess-retry path skip the ~60s neuronx
# compile. Must be set before jax initializes.
os.environ.setdefault(
    "JAX_COMPILATION_CACHE_DIR",
    os.path.join(tempfile.gettempdir(), "jaxcache"),
)

N = 1_000_000
C = 100
NCORES = 8
P = 128
W = 980
G = 49
CHUNKS = W // G      # 20
PECLS = int(os.environ.get("KV_PECLS", "79"))
_SLAB_ENDS = [int(v) for v in os.environ.get("KV_SLABS", "15,20").split(",")]
NSLAB = len(_SLAB_ENDS)
XBUFS = int(os.environ.get("KV_XBUFS", "3"))
EBUFS = int(os.environ.get("KV_EBUFS", "2"))
NPAD = NCORES * P * W
SENT = -300.0

_BOUNDS = np.linspace(0.0, 1.0, 11).astype(np.float32)

_built = {}


def _build_program():
    if "nc" in _built:
        return _built["nc"]

    import concourse.bacc as bacc
    import concourse.tile as tile
    from concourse import mybir

    f32 = mybir.dt.float32
    Alu = mybir.AluOpType
    Act = mybir.ActivationFunctionType
    AxX = mybir.AxisListType.X

    nc = bacc.Bacc("TRN2", target_bir_lowering=False, debug=False)
    x_d = nc.dram_tensor("x", [P, W * C], f32, kind="ExternalInput").ap()
    ident_d = nc.dram_tensor("ident", [P, P], f32, kind="ExternalInput").ap()
    nbnd_d = nc.dram_tensor("nbnd", [P, 11], f32, kind="ExternalInput").ap()
    acc_d = nc.dram_tensor("acc", [P, 33 * NSLAB], f32, kind="ExternalOutput").ap()

    slab_cols = [0] + [e * G for e in _SLAB_ENDS]
    assert slab_cols[-1] == W
    MAXSLAB = max(b - a for a, b in zip(slab_cols, slab_cols[1:]))

    with tile.TileContext(nc) as tc:
        with (
            tc.tile_pool(name="consts", bufs=1) as consts,
            tc.tile_pool(name="stats", bufs=1) as stats,
            tc.tile_pool(name="xin", bufs=XBUFS) as xin,
            tc.tile_pool(name="etmp", bufs=EBUFS) as etmp,
            tc.tile_pool(name="psum", bufs=2, space="PSUM") as psp,
        ):
            ident_t = consts.tile([P, P], f32)
            nc.gpsimd.dma_start(ident_t[:], ident_d[:, :])
            nbnd_t = consts.tile([P, 11], f32)
            nc.gpsimd.dma_start(nbnd_t[:], nbnd_d[:, :])

            M = stats.tile([P, W], f32, tag="M")
            S = stats.tile([P, W], f32, tag="S")
            SD = stats.tile([P, W], f32, tag="SD")
            XT = stats.tile([P, W], f32, tag="XT")
            corr = stats.tile([P, W], f32, tag="corr")
            ET = stats.tile([P, W], f32, tag="ET")
            RS = stats.tile([P, W], f32, tag="RS")
            PT = stats.tile([P, W], f32, tag="PT")
            Z = stats.tile([P, W], f32, tag="Z")
            ACC = stats.tile([P, 33 * NSLAB], f32, tag="ACC")
            junkW = stats.tile([P, MAXSLAB], f32, tag="junkW")
            junkR = stats.tile([P, MAXSLAB], f32, tag="junkR")

            def finish_slab(si):
                c0, c1 = slab_cols[si], slab_cols[si + 1]
                nc.vector.tensor_tensor(
                    corr[:, c0:c1], XT[:, c0:c1], M[:, c0:c1], op=Alu.is_equal
                )
                nc.scalar.activation(ET[:, c0:c1], XT[:, c0:c1], Act.Exp)
                nc.gpsimd.tensor_tensor(
                    S[:, c0:c1], S[:, c0:c1], SD[:, c0:c1], op=Alu.add
                )
                nc.vector.reciprocal(RS[:, c0:c1], S[:, c0:c1])
                nc.vector.tensor_tensor(
                    PT[:, c0:c1], ET[:, c0:c1], RS[:, c0:c1], op=Alu.mult
                )
                nc.gpsimd.tensor_tensor(
                    Z[:, c0:c1], PT[:, c0:c1], corr[:, c0:c1], op=Alu.mult
                )
                ab = 33 * si
                for b in range(11):
                    lo = float(_BOUNDS[b])
                    nw = c1 - c0
                    nc.vector.tensor_scalar(
                        junkW[:, :nw], PT[:, c0:c1], lo, None,
                        op0=Alu.is_gt, op1=Alu.add,
                        accum_out=ACC[:, ab + b:ab + b + 1],
                    )
                    nc.scalar.activation(
                        junkR[:, :nw], PT[:, c0:c1], Act.Relu,
                        bias=nbnd_t[:, b:b + 1],
                        accum_out=ACC[:, ab + 11 + b:ab + 12 + b],
                    )
                    nc.vector.tensor_scalar(
                        junkW[:, :nw], Z[:, c0:c1], lo, None,
                        op0=Alu.is_gt, op1=Alu.add,
                        accum_out=ACC[:, ab + 22 + b:ab + 23 + b],
                    )

            for k in range(CHUNKS):
                X = xin.tile([P, G * C], f32)
                nc.sync.dma_start(X[:], x_d[:, k * G * C:(k + 1) * G * C])
                x3 = X[:].rearrange("p (g c) -> p g c", c=C)
                nc.vector.tensor_reduce(
                    M[:, k * G:(k + 1) * G], x3, axis=AxX, op=Alu.max
                )
                nc.scalar.copy(
                    XT[:, k * G:(k + 1) * G],
                    x3[:, :, 0:1].rearrange("p g c -> p (g c)"),
                )
                E = etmp.tile([P, G * C], f32)
                nc.scalar.activation(E[:], X[:], Act.Exp)
                e3 = E[:].rearrange("p (g c) -> p g c", c=C)
                PS = psp.tile([P, G], f32)
                for cc in range(PECLS):
                    nc.tensor.matmul(
                        PS[:], ident_t[:],
                        e3[:, :, cc:cc + 1].rearrange("p g c -> p (g c)"),
                        start=(cc == 0), stop=(cc == PECLS - 1),
                    )
                nc.scalar.copy(S[:, k * G:(k + 1) * G], PS[:])
                nc.vector.tensor_reduce(
                    SD[:, k * G:(k + 1) * G], e3[:, :, PECLS:C],
                    axis=AxX, op=Alu.add,
                )
                if (k + 1) in _SLAB_ENDS:
                    finish_slab(_SLAB_ENDS.index(k + 1))

            nc.sync.dma_start(acc_d[:, :], ACC[:])

    nc.compile()
    _built["nc"] = nc
    return nc


def _prep_inputs(outputs, targets):
    """Sort rows by class, roll each row left by its class, pack densely."""
    x = np.ascontiguousarray(np.asarray(outputs, dtype=np.float32))
    t = np.asarray(targets).astype(np.int64).ravel()
    order = np.argsort(t, kind="stable")
    cnt = np.bincount(t, minlength=C)
    starts = np.zeros(C + 1, np.int64)
    starts[1:] = np.cumsum(cnt)

    Xr = np.empty((NPAD, C), np.float32)
    for c in range(C):
        s0, s1 = starts[c], starts[c + 1]
        if s1 == s0:
            continue
        src = x[order[s0:s1]]
        Xr[s0:s1, :C - c] = src[:, c:]
        if c:
            Xr[s0:s1, C - c:] = src[:, :c]
    Xr[N:] = 0.0
    Xr[N:, 0] = SENT

    Xv = Xr.reshape(NCORES, P, W * C)
    ident = np.eye(P, dtype=np.float32)
    nbnd = np.broadcast_to(-_BOUNDS, (P, 11)).copy()
    return [{"x": Xv[c], "ident": ident, "nbnd": nbnd} for c in range(NCORES)]


def _postprocess(acc_list):
    A = np.stack(acc_list)
    tot = A.astype(np.float64).sum(axis=(0, 1))
    tot = tot.reshape(NSLAB, 33).sum(axis=0)
    Cg, R, Zg = tot[0:11], tot[11:22], tot[22:33]
    bounds = _BOUNDS.astype(np.float64)
    SPcum = R + bounds * Cg                  # sum of p over {p > bound[b]}
    cnt = Cg[:10] - Cg[1:]
    sp = SPcum[:10] - SPcum[1:]
    sc = Zg[:10] - Zg[1:]
    nonempty = cnt > 0
    denom = np.where(nonempty, cnt, 1.0)
    ece = np.sum(np.where(nonempty, cnt * np.abs(sp / denom - sc / denom), 0.0))
    total = cnt.sum()
    val = ece / max(total, 1.0) if total > 0 else 0.0
    return np.float32(val)


def _exec(in_maps, trace=False):
    from concourse.bass_utils import run_bass_kernel_spmd

    nc = _build_program()
    res = run_bass_kernel_spmd(
        nc, in_maps, core_ids=list(range(NCORES)), trace=trace
    )
    return [res.results[c]["acc"] for c in range(NCORES)], res


def _subrun(tmpdir):
    """Subprocess entry: load prepped inputs, execute, save partials."""
    in_maps = []
    for c in range(NCORES):
        in_maps.append({
            "x": np.load(f"{tmpdir}/x{c}.npy"),
            "ident": np.load(f"{tmpdir}/ident.npy"),
            "nbnd": np.load(f"{tmpdir}/nbnd.npy"),
        })
    accs, _ = _exec(in_maps)
    np.save(f"{tmpdir}/accs.npy", np.stack(accs))


def _exec_subprocess(in_maps):
    """Run the device step in a fresh process (fresh PJRT client) — recovers
    from transient 'accelerator device unrecoverable' states."""
    import subprocess
    import tempfile

    here = os.path.dirname(os.path.abspath(__file__))
    with tempfile.TemporaryDirectory() as td:
        for c in range(NCORES):
            np.save(f"{td}/x{c}.npy", in_maps[c]["x"])
        np.save(f"{td}/ident.npy", in_maps[0]["ident"])
        np.save(f"{td}/nbnd.npy", in_maps[0]["nbnd"])
        code = (
            f"import sys; sys.path.insert(0, {here!r}); "
            f"import kernel; kernel._subrun({td!r})"
        )
        subprocess.run([sys.executable, "-c", code], check=True, timeout=2400)
        accs = np.load(f"{td}/accs.npy")
    return [accs[c] for c in range(NCORES)]


def _run(outputs, targets, trace=False):
    import time

    in_maps = _prep_inputs(outputs, targets)
    accs = None
    last_err = None
    try:
        accs, res = _exec(in_maps, trace=trace)
    except Exception as e:  # transient device-unrecoverable errors
        last_err = e
        res = None
        sys.stderr.write(f"kernel: in-process exec failed: {e}\n")
    if accs is None:
        for attempt in range(3):
            try:
                time.sleep(5.0)
                accs = _exec_subprocess(in_maps)
                break
            except Exception as e:
                last_err = e
                sys.stderr.write(
                    f"kernel: subprocess exec attempt {attempt} failed: {e}\n"
                )
        else:
            raise last_err
    val = _postprocess(accs)
    return val, res


def kernel(outputs, targets):
    val, _ = _run(outputs, targets, trace=False)
    return val



# revision 14
# speedup vs baseline: 1.6400x; 1.6400x over previous
"""Expected Calibration Error (histogram binning) on 8 Trainium2 NeuronCores.

kernel(outputs [1e6,100] f32, targets [1e6] int) -> f32 scalar, matching the
reference softmax/argmax/10-bin ECE. Data-parallel over the batch; each core
streams its shard once from HBM.

Layout trick (same as the f32 baseline): every per-row ECE statistic (row max,
row sum of exp, true-class prob, argmax==target, bin membership) is invariant
to a cyclic rotation of the row's class axis. The host rolls each row left by
its target class - the true-class logit lands in column 0 for every row - and
packs rows densely onto a [8 cores, 128 partitions, 980 rows] grid (pad rows
are [-300, 0, ...], whose exp underflows to 0 so p == 0 and the reference's
own p > 0 rule excludes them).

fp16 streaming: the host casts rolled logits to fp16, halving HBM traffic
(~70 us/core DMA). Measured off-device: the fp16 pipeline shifts the final
ECE by 6.0e-3 relative (18 of 1e6 rows flip argmax-tie correctness; the rest
is p-quantization) - well inside the 2e-2 gate.

Device, per chunk of 49 rows/partition ([128, 49, 100] fp16 tile):
  - DVE+Pool: row max via a pairwise tensor_tensor(max) tree (TensorReduce
    has no 2x mode; the tree runs at the DVE 2x fp16 rate, with the Pool
    engine taking a class-slice to offload DVE)
  - ACT:  exp of the whole tile (fp16 in/out) - the bottleneck engine
  - PE:   row-sum of exp for all 100 classes via accumulating fp16 identity
          matmuls (1 cycle/row vs 4 for f32)
  - DVE:  strided copies of column 0 (true-class logit + its exp)
Finish (interleaved slabs): p16 = exp(x_t) * (1/S) rounded fp16,
correct = (x_t == m), z = p * correct, then per boundary b:
  C_b = count(p16 > b)        (DVE is_gt at 4x fp16 + accumulate)
  R_b = sum(relu(p16 - b))    (DVE subtract/max, f32 junk + accumulate)
  Z_b = count(z16 > b)        (DVE is_gt at 4x fp16 + accumulate)
Host: sum 8x128 partials in f64; SP_b = R_b + b*C_b; adjacent differences
give the 10 bins; finish the ECE scalar exactly as the reference does.
"""

import os
import sys
import tempfile

import numpy as np

if "/opt/trn_rl_repo" not in sys.path:
    sys.path.insert(0, "/opt/trn_rl_repo")

# Persistent jax/PJRT executable cache (includes the compiled NEFF): makes
# repeat invocations and the subprocess-retry path skip the ~60s neuronx
# compile. Must be set before jax initializes.
os.environ.setdefault(
    "JAX_COMPILATION_CACHE_DIR",
    os.path.join(tempfile.gettempdir(), "jaxcache"),
)

N = 1_000_000
C = 100
NCORES = 8
P = 128
W = 980
G = 49
CHUNKS = W // G      # 20
_SLAB_ENDS = [int(v) for v in os.environ.get("KV_SLABS", "15,20").split(",")]
NSLAB = len(_SLAB_ENDS)
XBUFS = int(os.environ.get("KV_XBUFS", "4"))
EBUFS = int(os.environ.get("KV_EBUFS", "3"))
RJUNK16 = os.environ.get("KV_RJUNK16", "1") == "1"
NPAD = NCORES * P * W
SENT = -300.0

_BOUNDS = np.linspace(0.0, 1.0, 11).astype(np.float32)

_built = {}


def _tree_scratch_cols(nc_classes):
    """Total scratch columns (units of G) for a pairwise max tree."""
    cols = 0
    n = nc_classes
    while n > 1:
        m = n // 2
        cols += m
        n = m + (n % 2)
    return cols


def _build_program():
    if "nc" in _built:
        return _built["nc"]

    import concourse.bacc as bacc
    import concourse.tile as tile
    from concourse import mybir

    f16 = mybir.dt.float16
    f32 = mybir.dt.float32
    Alu = mybir.AluOpType
    Act = mybir.ActivationFunctionType

    nc = bacc.Bacc("TRN2", target_bir_lowering=False, debug=False)
    x_d = nc.dram_tensor("x", [P, W * C], f16, kind="ExternalInput").ap()
    ident_d = nc.dram_tensor("ident", [P, P], f16, kind="ExternalInput").ap()
    acc_d = nc.dram_tensor("acc", [P, 33 * NSLAB], f32, kind="ExternalOutput").ap()

    slab_cols = [0] + [e * G for e in _SLAB_ENDS]
    assert slab_cols[-1] == W
    MAXSLAB = max(b - a for a, b in zip(slab_cols, slab_cols[1:]))

    DCOLS = _tree_scratch_cols(C)

    with tile.TileContext(nc) as tc:
        with (
            tc.tile_pool(name="consts", bufs=1) as consts,
            tc.tile_pool(name="stats", bufs=1) as stats,
            tc.tile_pool(name="xin", bufs=XBUFS) as xin,
            tc.tile_pool(name="etmp", bufs=EBUFS) as etmp,
            tc.tile_pool(name="tree", bufs=2) as treep,
            tc.tile_pool(name="psum", bufs=2, space="PSUM") as psp,
        ):
            ident_t = consts.tile([P, P], f16)
            nc.gpsimd.dma_start(ident_t[:], ident_d[:, :])

            M16 = stats.tile([P, W], f16, tag="M16")
            XT16 = stats.tile([P, W], f16, tag="XT16")
            ET16 = stats.tile([P, W], f16, tag="ET16")
            corr = stats.tile([P, W], f16, tag="corr")
            S32 = stats.tile([P, W], f32, tag="S32")
            RS32 = stats.tile([P, W], f32, tag="RS32")
            PT16 = stats.tile([P, W], f16, tag="PT16")
            Z16 = stats.tile([P, W], f16, tag="Z16")
            ACC = stats.tile([P, 33 * NSLAB], f32, tag="ACC")
            junkC = stats.tile([P, MAXSLAB], f16, tag="junkC")
            junkZ = stats.tile([P, MAXSLAB], f16, tag="junkZ")
            junkR = stats.tile([P, MAXSLAB], f16, tag="junkR")

            def max_tree(eng, x3, scratch, off, c0, c1, out):
                """Pairwise max over classes [c0,c1) of x3 into out [P, G]."""
                cur = x3[:, :, c0:c1]
                n = c1 - c0
                carries = []
                while n > 1:
                    m = n // 2
                    if n % 2:
                        carries.append(cur[:, :, n - 1:n])
                    out3 = scratch[:, off * G:(off + m) * G].rearrange(
                        "p (g c) -> p g c", c=m
                    )
                    eng.tensor_tensor(
                        out3, cur[:, :, 0:m], cur[:, :, m:2 * m], op=Alu.max
                    )
                    cur = out3
                    off += m
                    n = m
                res = cur.rearrange("p g c -> p (g c)")
                if not carries:
                    eng.tensor_copy(out, res)
                    return
                for ca in carries[:-1]:
                    eng.tensor_tensor(
                        res, res, ca.rearrange("p g c -> p (g c)"), op=Alu.max
                    )
                eng.tensor_tensor(
                    out, res, carries[-1].rearrange("p g c -> p (g c)"),
                    op=Alu.max,
                )

            def finish_slab(si):
                c0, c1 = slab_cols[si], slab_cols[si + 1]
                nw = c1 - c0
                nc.vector.tensor_tensor(
                    corr[:, c0:c1], XT16[:, c0:c1], M16[:, c0:c1],
                    op=Alu.is_equal,
                )
                nc.vector.reciprocal(RS32[:, c0:c1], S32[:, c0:c1])
                nc.vector.tensor_tensor(
                    PT16[:, c0:c1], ET16[:, c0:c1], RS32[:, c0:c1], op=Alu.mult
                )
                nc.gpsimd.tensor_tensor(
                    Z16[:, c0:c1], PT16[:, c0:c1], corr[:, c0:c1], op=Alu.mult
                )
                ab = 33 * si
                for b in range(11):
                    lo = float(_BOUNDS[b])
                    nc.vector.tensor_scalar(
                        junkC[:, :nw], PT16[:, c0:c1], lo, None,
                        op0=Alu.is_gt, op1=Alu.add,
                        accum_out=ACC[:, ab + b:ab + b + 1],
                    )
                    # accum's reduce op follows op1 (op1=max would
                    # max-reduce), so sum max(p,b) and let the host subtract
                    # the exact b*(n - C_b) of the clamped elements; the
                    # accumulator sums pre-rounding f32 values (measured).
                    nc.vector.tensor_scalar(
                        junkR[:, :nw], PT16[:, c0:c1], lo, None,
                        op0=Alu.max, op1=Alu.add,
                        accum_out=ACC[:, ab + 11 + b:ab + 12 + b],
                    )
                    nc.vector.tensor_scalar(
                        junkZ[:, :nw], Z16[:, c0:c1], lo, None,
                        op0=Alu.is_gt, op1=Alu.add,
                        accum_out=ACC[:, ab + 22 + b:ab + 23 + b],
                    )

            for k in range(CHUNKS):
                X = xin.tile([P, G * C], f16)
                nc.sync.dma_start(X[:], x_d[:, k * G * C:(k + 1) * G * C])
                x3 = X[:].rearrange("p (g c) -> p g c", c=C)

                scratch = treep.tile([P, DCOLS * G], f16)
                max_tree(nc.vector, x3, scratch[:], 0, 0, C,
                         M16[:, k * G:(k + 1) * G])
                nc.gpsimd.tensor_copy(
                    XT16[:, k * G:(k + 1) * G],
                    x3[:, :, 0:1].rearrange("p g c -> p (g c)"),
                )

                E = etmp.tile([P, G * C], f16)
                nc.scalar.activation(E[:], X[:], Act.Exp)
                e3 = E[:].rearrange("p (g c) -> p g c", c=C)
                nc.gpsimd.tensor_copy(
                    ET16[:, k * G:(k + 1) * G],
                    e3[:, :, 0:1].rearrange("p g c -> p (g c)"),
                )

                PS = psp.tile([P, G], f32)
                for cc in range(C):
                    nc.tensor.matmul(
                        PS[:], ident_t[:],
                        e3[:, :, cc:cc + 1].rearrange("p g c -> p (g c)"),
                        start=(cc == 0), stop=(cc == C - 1),
                    )
                nc.vector.tensor_copy(S32[:, k * G:(k + 1) * G], PS[:])

                if (k + 1) in _SLAB_ENDS:
                    finish_slab(_SLAB_ENDS.index(k + 1))

            nc.sync.dma_start(acc_d[:, :], ACC[:])

    nc.compile()
    _built["nc"] = nc
    return nc


def _prep_inputs(outputs, targets):
    """Sort rows by class, roll each row left by its class, pack densely."""
    x = np.ascontiguousarray(np.asarray(outputs, dtype=np.float32))
    t = np.asarray(targets).astype(np.int64).ravel()
    order = np.argsort(t, kind="stable")
    cnt = np.bincount(t, minlength=C)
    starts = np.zeros(C + 1, np.int64)
    starts[1:] = np.cumsum(cnt)

    Xr = np.empty((NPAD, C), np.float16)
    for c in range(C):
        s0, s1 = starts[c], starts[c + 1]
        if s1 == s0:
            continue
        src = x[order[s0:s1]]
        Xr[s0:s1, :C - c] = src[:, c:]
        if c:
            Xr[s0:s1, C - c:] = src[:, :c]
    Xr[N:] = 0.0
    Xr[N:, 0] = SENT

    Xv = Xr.reshape(NCORES, P, W * C)
    ident = np.eye(P, dtype=np.float16)
    return [{"x": Xv[c], "ident": ident} for c in range(NCORES)]


def _postprocess(acc_list):
    A = np.stack(acc_list)
    tot = A.astype(np.float64).sum(axis=(0, 1))
    tot = tot.reshape(NSLAB, 33).sum(axis=0)
    Cg, MX, Zg = tot[0:11], tot[11:22], tot[22:33]
    bounds = _BOUNDS.astype(np.float64)
    # MX_b = sum(max(p, b)); elements with p <= b contributed exactly b each
    SPcum = MX - bounds * (NPAD - Cg)        # sum of p over {p > bound[b]}
    cnt = Cg[:10] - Cg[1:]
    sp = SPcum[:10] - SPcum[1:]
    sc = Zg[:10] - Zg[1:]
    nonempty = cnt > 0
    denom = np.where(nonempty, cnt, 1.0)
    ece = np.sum(np.where(nonempty, cnt * np.abs(sp / denom - sc / denom), 0.0))
    total = cnt.sum()
    val = ece / max(total, 1.0) if total > 0 else 0.0
    return np.float32(val)


def _exec(in_maps, trace=False):
    from concourse.bass_utils import run_bass_kernel_spmd

    nc = _build_program()
    res = run_bass_kernel_spmd(
        nc, in_maps, core_ids=list(range(NCORES)), trace=trace
    )
    return [res.results[c]["acc"] for c in range(NCORES)], res


def _subrun(tmpdir):
    """Subprocess entry: load prepped inputs, execute, save partials."""
    in_maps = []
    for c in range(NCORES):
        in_maps.append({
            "x": np.load(f"{tmpdir}/x{c}.npy"),
            "ident": np.load(f"{tmpdir}/ident.npy"),
        })
    accs, _ = _exec(in_maps)
    np.save(f"{tmpdir}/accs.npy", np.stack(accs))


def _exec_subprocess(in_maps):
    """Run the device step in a fresh process (fresh PJRT client) - recovers
    from transient 'accelerator device unrecoverable' states."""
    import subprocess
    import tempfile

    here = os.path.dirname(os.path.abspath(__file__))
    with tempfile.TemporaryDirectory() as td:
        for c in range(NCORES):
            np.save(f"{td}/x{c}.npy", in_maps[c]["x"])
        np.save(f"{td}/ident.npy", in_maps[0]["ident"])
        code = (
            f"import sys; sys.path.insert(0, {here!r}); "
            f"import kernel; kernel._subrun({td!r})"
        )
        subprocess.run([sys.executable, "-c", code], check=True, timeout=2400)
        accs = np.load(f"{td}/accs.npy")
    return [accs[c] for c in range(NCORES)]


def _run(outputs, targets, trace=False):
    import time

    in_maps = _prep_inputs(outputs, targets)
    accs = None
    last_err = None
    try:
        accs, res = _exec(in_maps, trace=trace)
    except Exception as e:  # transient device-unrecoverable errors
        last_err = e
        res = None
        sys.stderr.write(f"kernel: in-process exec failed: {e}\n")
    if accs is None:
        for attempt in range(3):
            try:
                time.sleep(5.0)
                accs = _exec_subprocess(in_maps)
                break
            except Exception as e:
                last_err = e
                sys.stderr.write(
                    f"kernel: subprocess exec attempt {attempt} failed: {e}\n"
                )
        else:
            raise last_err
    val = _postprocess(accs)
    return val, res


def kernel(outputs, targets):
    val, _ = _run(outputs, targets, trace=False)
    return val
